# revision 2
# baseline (speedup 1.0000x reference)
"""Multi-head cross attention on 8 trn2 NeuronCores.

Problem: B=2, T=4096, EMB=512, H=8 heads (head dim 64), fp32 I/O.
  q = x1 @ Wq.T + bq ; k,v from x2 ; S = q k^T / sqrt(512) ;
  softmax over keys with -1e10 masking ; out = (A v) @ Wu.T + bu.

Sharding: core c handles batch b = c//4 and query rows
[1024*(c%4), 1024*(c%4+1)).  Each core computes K,V for its batch in
full (4-way duplication), its own Q chunk, attention, and out-proj.

Device-side layout choices:
  - All matmul operands fp16 (PE rate is dtype-independent; fp16 halves
    DMA/SBUF and keeps ~1e-3 accuracy), accumulation fp32 in PSUM.
  - Scores computed TRANSPOSED, S^T[key, query]: contraction over the
    head dim requires Q^T/K^T (head-dim on partitions), which fall out
    of computing the projections transposed from x^T inputs (host
    pre-transposes x1/x2/W).  With keys on partitions, P^T = exp(S^T)*M^T
    feeds the AV matmul directly as its stationary-side contraction
    without any on-chip transposes.
  - Scores are small (|S| < ~1) so exp needs no max-subtraction; the
    1/sqrt(512) scale is folded into the ACT exp instruction.
  - V is stored interleaved [key, head, 65] with a ones column so the
    AV matmul also produces the softmax denominators r[q] (row 64).
  - Normalization is deferred: Y^T_h / r_h via reciprocal + a K=1
    broadcast matmul + one DVE multiply per (head, chunk).
  - 2 heads are packed per scores pass via tile_position row-tiling
    (contraction=64 -> rows 0-63 / 64-127 run concurrently).
"""
import math
import os
from contextlib import ExitStack

import numpy as np

import concourse.bass as bass
import concourse.bacc as bacc
import concourse.tile as tile
import concourse.mybir as mybir
from concourse.bass_utils import run_bass_kernel_spmd

F16 = mybir.dt.float16
F32 = mybir.dt.float32
EXP = mybir.ActivationFunctionType.Exp

EMB, H, D, CT = 512, 8, 64, 4  # emb, heads, head dim, emb/128

FULL_CFG = dict(T=4096, QC=1024)  # keys per batch, query rows per core
MINI_CFG = dict(T=512, QC=256)


def attention_body(ctx, tc, io, cfg):
    nc = tc.nc
    T, QC = cfg["T"], cfg["QC"]
    KT = T // 128            # key tiles
    NG = KT // 2             # key-tile groups of 2
    CH = min(512, QC)        # query chunk width
    NCH = QC // CH
    scale = 1.0 / math.sqrt(EMB)

    pw = ctx.enter_context(tc.tile_pool(name="w", bufs=1))
    pk = ctx.enter_context(tc.tile_pool(name="kt", bufs=1))
    pv = ctx.enter_context(tc.tile_pool(name="v", bufs=1))
    pq = ctx.enter_context(tc.tile_pool(name="qt", bufs=1))

    # persistent weights / biases / constants
    wq = [pw.tile([128, EMB], F16, tag=f"wq{i}", name=f"wq{i}") for i in range(CT)]
    wk = [pw.tile([128, EMB], F16, tag=f"wk{i}", name=f"wk{i}") for i in range(CT)]
    wv = [pw.tile([128, EMB], F16, tag=f"wv{i}", name=f"wv{i}") for i in range(CT)]
    wu = [pw.tile([128, EMB], F16, tag=f"wu{i}", name=f"wu{i}") for i in range(CT)]
    for i in range(CT):
        nc.sync.dma_start(wq[i][:], io["wqT"][bass.ts(i, 128), :])
        nc.sync.dma_start(wk[i][:], io["wkT"][bass.ts(i, 128), :])
        nc.sync.dma_start(wv[i][:], io["wvT"][bass.ts(i, 128), :])
        nc.sync.dma_start(wu[i][:], io["wuT"][bass.ts(i, 128), :])
    bqr = pw.tile([128, CT], F32, tag="bqr", name="bqr")
    bkr = pw.tile([128, CT], F32, tag="bkr", name="bkr")
    bvb = pw.tile([128, EMB], F32, tag="bvb", name="bvb")
    bub = pw.tile([128, EMB], F32, tag="bub", name="bub")
    nc.sync.dma_start(bqr[:], io["bqr"][:, :])
    nc.sync.dma_start(bkr[:], io["bkr"][:, :])
    nc.sync.dma_start(bvb[:], io["bvb"][:, :])
    nc.sync.dma_start(bub[:], io["bub"][:, :])
    ones = pw.tile([1, D], F16, tag="ones", name="ones")
    nc.vector.memset(ones[:], 1.0)

    # persistent K^T [emb, T], V [T, head, 65(+pad)], Q^T [emb, QC]
    kt = [pk.tile([128, T], F16, tag=f"kt{i}", name=f"kt{i}") for i in range(CT)]
    v = pv.tile([128, KT, H, 66], F16, tag="v", name="v")
    nc.vector.memset(v[:, :, :, 64:65], 1.0)
    qt = [pq.tile([128, QC], F16, tag=f"qt{i}", name=f"qt{i}") for i in range(CT)]

    with tc.tile_pool(name="x", bufs=1) as px, \
         tc.tile_pool(name="pp", bufs=4, space="PSUM") as pp:
        x2t = [px.tile([128, T], F16, tag=f"x2t{i}", name=f"x2t{i}") for i in range(CT)]
        x1t = [px.tile([128, QC], F16, tag=f"x1t{i}", name=f"x1t{i}") for i in range(CT)]
        for i in range(CT):
            nc.sync.dma_start(x2t[i][:], io["x2T"][bass.ts(i, 128), :])
            nc.sync.dma_start(x1t[i][:], io["x1T"][bass.ts(i, 128), :])

        # K^T[e,t] = sum_c WkT[c,e] * x2T[c,t]  (+ bk per-partition)
        for e in range(CT):
            for t in range(T // 512):
                ps = pp.tile([128, 512], F32, tag="ps", name="ps")
                for c in range(CT):
                    nc.tensor.matmul(ps[:], wk[c][:, bass.ts(e, 128)],
                                     x2t[c][:, bass.ts(t, 512)],
                                     start=(c == 0), stop=(c == CT - 1))
                nc.vector.tensor_scalar_add(kt[e][:, bass.ts(t, 512)], ps[:],
                                            bkr[:, e:e + 1])
        # Q^T[e,q] likewise from x1T
        for e in range(CT):
            for t in range(QC // CH):
                ps = pp.tile([128, CH], F32, tag="ps2", name="ps2")
                for c in range(CT):
                    nc.tensor.matmul(ps[:], wq[c][:, bass.ts(e, 128)],
                                     x1t[c][:, bass.ts(t, CH)],
                                     start=(c == 0), stop=(c == CT - 1))
                nc.vector.tensor_scalar_add(qt[e][:, bass.ts(t, CH)], ps[:],
                                            bqr[:, e:e + 1])
        # V[t,e] = sum_c x2T[c,t] * WvT[c,e] (+ bv), interleaved per head
        for t in range(KT):
            ps = pp.tile([128, EMB], F32, tag="ps", name="psv")
            for c in range(CT):
                nc.tensor.matmul(ps[:], x2t[c][:, bass.ts(t, 128)], wv[c][:],
                                 start=(c == 0), stop=(c == CT - 1))
            nc.vector.tensor_add(
                v[:, t, :, 0:64],
                ps[:].rearrange("p (h d) -> p h d", h=H),
                bvb[:].rearrange("p (h d) -> p h d", h=H))

    # attention
    with tc.tile_pool(name="ps_s", bufs=1, space="PSUM") as ps_s, \
         tc.tile_pool(name="ps_av", bufs=3, space="PSUM") as ps_av, \
         tc.tile_pool(name="ps_b", bufs=1, space="PSUM") as ps_b, \
         tc.tile_pool(name="pe", bufs=2) as pe, \
         tc.tile_pool(name="ppp", bufs=2) as ppp, \
         tc.tile_pool(name="pm", bufs=NG) as pm, \
         tc.tile_pool(name="py", bufs=CT) as py, \
         tc.tile_pool(name="pys", bufs=2) as pys, \
         tc.tile_pool(name="prr", bufs=2) as prr, \
         tc.tile_pool(name="po", bufs=2) as po:
        for ch in range(NCH):
            mk = []
            for g in range(NG):
                m = pm.tile([128, 2 * CH], F16, tag="mk", name="mk")
                for i in range(2):
                    nc.sync.dma_start(
                        m[:, bass.ts(i, CH)],
                        io["maskT"][bass.ds(128 * (2 * g + i), 128),
                                    bass.ds(CH * ch, CH)])
                mk.append(m)
            yts = [py.tile([128, CH], F16, tag="yt", name=f"yt{e}")
                   for e in range(CT)]
            for pr in range(CT):  # head pair
                av = [ps_av.tile([65, CH], F32, tag="av", name="av")
                      for _ in range(2)]
                for g in range(NG):
                    ps = ps_s.tile([128, 4 * CH], F32, tag="s", name="ps_s")
                    for i in range(2):
                        kk = 2 * g + i
                        for hh in range(2):  # head within pair
                            nc.tensor.matmul(
                                ps[:, bass.ds(CH * (2 * hh + i), CH)],
                                kt[pr][bass.ds(64 * hh, 64), bass.ts(kk, 128)],
                                qt[pr][bass.ds(64 * hh, 64), bass.ds(CH * ch, CH)],
                                start=True, stop=True,
                                tile_position=(64 * hh, 0))
                    e16 = pe.tile([128, 4 * CH], F16, tag="E", name="e16")
                    nc.scalar.activation(e16[:], ps[:], EXP, scale=scale)
                    pt = ppp.tile([128, 4 * CH], F16, tag="P", name="pt")
                    nc.vector.tensor_mul(pt[:, 0:2 * CH], e16[:, 0:2 * CH], mk[g][:])
                    nc.vector.tensor_mul(pt[:, 2 * CH:4 * CH], e16[:, 2 * CH:4 * CH],
                                         mk[g][:])
                    for i in range(2):
                        kk = 2 * g + i
                        for hh in range(2):
                            nc.tensor.matmul(
                                av[hh][:], v[:, kk, 2 * pr + hh, 0:65],
                                pt[:, bass.ds(CH * (2 * hh + i), CH)],
                                start=(kk == 0), stop=(kk == KT - 1))
                for hh in range(2):
                    ysb = pys.tile([65, CH], F32, tag="ys", name="ysb")
                    nc.scalar.copy(ysb[:], av[hh][:])
                    rr = prr.tile([1, CH], F16, tag="rr", name="rr")
                    with nc.allow_low_precision(reason="r~2e3, fp16 recip ok"):
                        nc.vector.reciprocal(rr[:], ysb[64:65, :])
                    bc = ps_b.tile([64, CH], F32, tag="bc", name="bc")
                    nc.tensor.matmul(bc[:], ones[:], rr[:], start=True, stop=True)
                    nc.vector.tensor_mul(yts[pr][bass.ds(64 * hh, 64), :],
                                         ysb[0:64, :], bc[:])
            # out[q, :] = sum_e Y^T[e, q] * WuT[e, :] + bu
            for qi in range(CH // 128):
                pso = ps_av.tile([128, EMB], F32, tag="av", name="pso")
                for e in range(CT):
                    nc.tensor.matmul(pso[:], yts[e][:, bass.ts(qi, 128)], wu[e][:],
                                     start=(e == 0), stop=(e == CT - 1))
                osb = po.tile([128, EMB], F32, tag="o", name="osb")
                nc.vector.tensor_add(osb[:], pso[:], bub[:])
                nc.sync.dma_start(
                    io["out"][bass.ds(CH * ch + 128 * qi, 128), :], osb[:])


def build(cfg, num_devices=8):
    T, QC = cfg["T"], cfg["QC"]
    nc = bacc.Bacc("TRN2", target_bir_lowering=False, debug=False,
                   num_devices=num_devices)
    io = {
        "x1T": nc.dram_tensor("x1T", [EMB, QC], F16, kind="ExternalInput").ap(),
        "x2T": nc.dram_tensor("x2T", [EMB, T], F16, kind="ExternalInput").ap(),
        "maskT": nc.dram_tensor("maskT", [T, QC], F16, kind="ExternalInput").ap(),
        "wqT": nc.dram_tensor("wqT", [EMB, EMB], F16, kind="ExternalInput").ap(),
        "wkT": nc.dram_tensor("wkT", [EMB, EMB], F16, kind="ExternalInput").ap(),
        "wvT": nc.dram_tensor("wvT", [EMB, EMB], F16, kind="ExternalInput").ap(),
        "wuT": nc.dram_tensor("wuT", [EMB, EMB], F16, kind="ExternalInput").ap(),
        "bqr": nc.dram_tensor("bqr", [128, CT], F32, kind="ExternalInput").ap(),
        "bkr": nc.dram_tensor("bkr", [128, CT], F32, kind="ExternalInput").ap(),
        "bvb": nc.dram_tensor("bvb", [128, EMB], F32, kind="ExternalInput").ap(),
        "bub": nc.dram_tensor("bub", [128, EMB], F32, kind="ExternalInput").ap(),
        "out": nc.dram_tensor("out", [QC, EMB], F32, kind="ExternalOutput").ap(),
    }
    with tile.TileContext(nc) as tc:
        with ExitStack() as ctx:
            attention_body(ctx, tc, io, cfg)
    nc.compile()
    return nc


def host_prep(x1, x2, mask, Wq, bq, Wk, bk, Wv, bv, Wu, bu, cfg):
    """Build the 8 per-core input maps from full inputs."""
    T, QC = cfg["T"], cfg["QC"]
    shared = {
        "wqT": np.ascontiguousarray(Wq.T).astype(np.float16),
        "wkT": np.ascontiguousarray(Wk.T).astype(np.float16),
        "wvT": np.ascontiguousarray(Wv.T).astype(np.float16),
        "wuT": np.ascontiguousarray(Wu.T).astype(np.float16),
        "bqr": np.ascontiguousarray(bq.reshape(CT, 128).T).astype(np.float32),
        "bkr": np.ascontiguousarray(bk.reshape(CT, 128).T).astype(np.float32),
        "bvb": np.ascontiguousarray(np.broadcast_to(bv, (128, EMB))).astype(np.float32),
        "bub": np.ascontiguousarray(np.broadcast_to(bu, (128, EMB))).astype(np.float32),
    }
    x2T = [x2[b].T.astype(np.float16) for b in range(x1.shape[0])]
    in_maps = []
    n_cores = (x1.shape[0] * x1.shape[1]) // QC
    per_b = x1.shape[1] // QC
    for c in range(n_cores):
        b, q0 = c // per_b, (c % per_b) * QC
        in_maps.append(dict(
            shared,
            x1T=x1[b, q0:q0 + QC, :].T.astype(np.float16),
            x2T=x2T[b],
            maskT=mask[b, q0:q0 + QC, :].T.astype(np.float16),
        ))
    return in_maps


_NC_CACHE = {}


def kernel(x1, x2, mask, Wq, bq, Wk, bk, Wv, bv, Wu, bu):
    cfg = FULL_CFG
    B, TQ, _ = x1.shape
    in_maps = host_prep(np.asarray(x1, np.float32), np.asarray(x2, np.float32),
                        np.asarray(mask), np.asarray(Wq, np.float32),
                        np.asarray(bq, np.float32), np.asarray(Wk, np.float32),
                        np.asarray(bk, np.float32), np.asarray(Wv, np.float32),
                        np.asarray(bv, np.float32), np.asarray(Wu, np.float32),
                        np.asarray(bu, np.float32), cfg)
    key = (cfg["T"], cfg["QC"])
    if key not in _NC_CACHE:
        _NC_CACHE[key] = build(cfg)
    nc = _NC_CACHE[key]
    res = run_bass_kernel_spmd(nc, in_maps, core_ids=list(range(8)),
                               trace=bool(os.environ.get("KERNEL_TRACE")))
    if os.environ.get("KERNEL_TRACE"):
        kernel.last_exec_ns = res.exec_time_ns
        kernel.last_results = res
    out = np.empty((B, TQ, EMB), np.float32)
    per_b = TQ // cfg["QC"]
    for c in range(8):
        b, q0 = c // per_b, (c % per_b) * cfg["QC"]
        out[b, q0:q0 + cfg["QC"], :] = res.results[c]["out"]
    return out


# revision 11
# speedup vs baseline: 1.2760x; 1.2760x over previous
"""Multi-head cross attention on 8 trn2 NeuronCores.

Problem: B=2, T=4096, EMB=512, H=8 heads (head dim 64), fp32 I/O.
  q = x1 @ Wq.T + bq ; k,v from x2 ; S = q k^T / sqrt(512) ;
  softmax over keys with -1e10 masking ; out = (A v) @ Wu.T + bu.

Sharding: core c handles batch b = c//4 and query rows
[1024*(c%4), 1024*(c%4+1)).  Each core computes K,V for its batch in
full (4-way duplication), its own Q chunk, attention, and out-proj.

Device-side layout choices:
  - All matmul operands fp16 (PE rate is dtype-independent; fp16 halves
    DMA/SBUF and keeps ~1e-3 accuracy), accumulation fp32 in PSUM.
  - Scores computed TRANSPOSED, S^T[key, query]: contraction over the
    head dim requires Q^T/K^T (head-dim on partitions), which fall out
    of computing the projections transposed from x^T inputs (host
    pre-transposes x1/x2/W).  With keys on partitions, P^T = exp(S^T)*M^T
    feeds the AV matmul directly as its stationary-side contraction
    without any on-chip transposes.
  - Scores are small (|S| < ~1) so exp needs no max-subtraction; the
    1/sqrt(512) scale is folded into the ACT exp instruction.
  - V is stored interleaved [key, head, 65] with a ones column so the
    AV matmul also produces the softmax denominators r[q] (row 64).
  - Normalization is deferred: Y^T_h / r_h via reciprocal + a K=1
    broadcast matmul + one DVE multiply per (head, chunk).
  - 2 heads are packed per scores pass via tile_position row-tiling
    (contraction=64 -> rows 0-63 / 64-127 run concurrently).
"""
import math
import os
from contextlib import ExitStack

import numpy as np

import concourse.bass as bass
import concourse.bacc as bacc
import concourse.tile as tile
import concourse.mybir as mybir
from concourse.bass_utils import run_bass_kernel_spmd

F16 = mybir.dt.float16
F32 = mybir.dt.float32
EXP = mybir.ActivationFunctionType.Exp

EMB, H, D, CT = 512, 8, 64, 4  # emb, heads, head dim, emb/128

FULL_CFG = dict(T=4096, QC=1024)  # keys per batch, query rows per core
MINI_CFG = dict(T=512, QC=256)


def attention_body(ctx, tc, io, cfg):
    nc = tc.nc
    T, QC = cfg["T"], cfg["QC"]
    KT = T // 128            # key tiles
    NG = KT // 2             # key-tile groups of 2
    CH = min(512, QC)        # query chunk width
    NCH = QC // CH
    scale = 1.0 / math.sqrt(EMB)

    pw = ctx.enter_context(tc.tile_pool(name="w", bufs=1))
    pk = ctx.enter_context(tc.tile_pool(name="kt", bufs=1))
    pv = ctx.enter_context(tc.tile_pool(name="v", bufs=1))
    pq = ctx.enter_context(tc.tile_pool(name="qt", bufs=1))

    # persistent weights / biases / constants
    wq = [pw.tile([128, EMB], F16, tag=f"wq{i}", name=f"wq{i}") for i in range(CT)]
    wk = [pw.tile([128, EMB], F16, tag=f"wk{i}", name=f"wk{i}") for i in range(CT)]
    wv = [pw.tile([128, EMB], F16, tag=f"wv{i}", name=f"wv{i}") for i in range(CT)]
    wu = [pw.tile([128, EMB], F16, tag=f"wu{i}", name=f"wu{i}") for i in range(CT)]
    for i in range(CT):
        nc.sync.dma_start(wq[i][:], io["wqT"][bass.ts(i, 128), :])
        nc.sync.dma_start(wk[i][:], io["wkT"][bass.ts(i, 128), :])
        nc.sync.dma_start(wv[i][:], io["wvT"][bass.ts(i, 128), :])
        nc.sync.dma_start(wu[i][:], io["wuT"][bass.ts(i, 128), :])
    bqr = pw.tile([128, CT], F32, tag="bqr", name="bqr")
    bkr = pw.tile([128, CT], F32, tag="bkr", name="bkr")
    bvb = pw.tile([128, EMB], F32, tag="bvb", name="bvb")
    bub = pw.tile([128, EMB], F32, tag="bub", name="bub")
    nc.sync.dma_start(bqr[:], io["bqr"][:, :])
    nc.sync.dma_start(bkr[:], io["bkr"][:, :])
    nc.sync.dma_start(bvb[:], io["bvb"][:, :])
    nc.sync.dma_start(bub[:], io["bub"][:, :])
    ones = pw.tile([1, D], F16, tag="ones", name="ones")
    nc.vector.memset(ones[:], 1.0)

    # persistent K^T [emb, T], V [T, head, 65(+pad)], Q^T [emb, QC]
    kt = [pk.tile([128, T], F16, tag=f"kt{i}", name=f"kt{i}") for i in range(CT)]
    v = pv.tile([128, KT, H, 66], F16, tag="v", name="v")
    nc.vector.memset(v[:, :, :, 64:65], 1.0)
    qt = [pq.tile([128, QC], F16, tag=f"qt{i}", name=f"qt{i}") for i in range(CT)]

    with tc.tile_pool(name="x", bufs=1) as px, \
         tc.tile_pool(name="pp", bufs=4, space="PSUM") as pp:
        x2t = [px.tile([128, T], F16, tag=f"x2t{i}", name=f"x2t{i}") for i in range(CT)]
        x1t = [px.tile([128, QC], F16, tag=f"x1t{i}", name=f"x1t{i}") for i in range(CT)]
        for i in range(CT):
            nc.sync.dma_start(x1t[i][:], io["x1T"][bass.ts(i, 128), :])
            for hf in range(2):
                nc.sync.dma_start(x2t[i][:, bass.ts(hf, T // 2)],
                                  io["x2T"][bass.ts(i, 128), bass.ts(hf, T // 2)])

        # Q^T[e,q] = sum_c WqT[c,e] * x1T[c,q]  (+ bq per-partition)
        for e in range(CT):
            for t in range(QC // CH):
                ps = pp.tile([128, CH], F32, tag="ps2", name="ps2")
                for c in range(CT):
                    nc.tensor.matmul(ps[:], wq[c][:, bass.ts(e, 128)],
                                     x1t[c][:, bass.ts(t, CH)],
                                     start=(c == 0), stop=(c == CT - 1))
                nc.vector.tensor_scalar_add(qt[e][:, bass.ts(t, CH)], ps[:],
                                            bqr[:, e:e + 1])
        # K^T[e,t] = sum_c WkT[c,e] * x2T[c,t] (+ bk); V[t,e] interleaved
        for e in range(CT):
            for t in range(T // 512):
                ps = pp.tile([128, 512], F32, tag="ps", name="ps")
                for c in range(CT):
                    nc.tensor.matmul(ps[:], wk[c][:, bass.ts(e, 128)],
                                     x2t[c][:, bass.ts(t, 512)],
                                     start=(c == 0), stop=(c == CT - 1))
                nc.vector.tensor_scalar_add(kt[e][:, bass.ts(t, 512)], ps[:],
                                            bkr[:, e:e + 1])
            if e > 0:
                continue
            # V right after K^T e-tile 0 so attention pair 0 can start
            for t in range(KT):
                ps = pp.tile([128, EMB], F32, tag="ps", name="psv")
                for c in range(CT):
                    nc.tensor.matmul(ps[:], x2t[c][:, bass.ts(t, 128)], wv[c][:],
                                     start=(c == 0), stop=(c == CT - 1))
                nc.vector.tensor_add(
                    v[:, t, :, 0:64],
                    ps[:].rearrange("p (h d) -> p h d", h=H),
                    bvb[:].rearrange("p (h d) -> p h d", h=H))

    # optional debug dumps of intermediates
    if "dbg" in io:
        for e in range(CT):
            nc.sync.dma_start(io["dbg_qt"][bass.ts(e, 128), :], qt[e][:])
            nc.sync.dma_start(io["dbg_kt"][bass.ts(e, 128), :], kt[e][:])
        for t in range(KT):
            nc.sync.dma_start(
                io["dbg_v"][:, :].rearrange("p (a b) -> p a b", a=KT)[:, t, :],
                v[:, t, :, :].rearrange("p a b -> p (a b)"))

    # attention
    with tc.tile_pool(name="ps_s", bufs=2, space="PSUM") as ps_s, \
         tc.tile_pool(name="ps_av", bufs=2, space="PSUM") as ps_av, \
         tc.tile_pool(name="ps_b", bufs=2, space="PSUM") as ps_b, \
         tc.tile_pool(name="pe", bufs=2) as pe, \
         tc.tile_pool(name="ppp", bufs=2) as ppp, \
         tc.tile_pool(name="pm", bufs=KT) as pm, \
         tc.tile_pool(name="py", bufs=CT) as py, \
         tc.tile_pool(name="pys", bufs=2) as pys, \
         tc.tile_pool(name="prr", bufs=2) as prr, \
         tc.tile_pool(name="po", bufs=2) as po:
        for ch in range(NCH):
            mk = []
            for kk in range(KT):
                m = pm.tile([128, CH], F16, tag="mk", name="mk")
                nc.sync.dma_start(
                    m[:], io["maskT"][bass.ts(kk, 128), bass.ds(CH * ch, CH)])
                mk.append(m)
            yts = [py.tile([128, CH], F16, tag="yt", name=f"yt{e}")
                   for e in range(CT)]
            for pr in range(CT):  # head pair
                av = [ps_av.tile([65, CH], F32, tag="av", name="av")
                      for _ in range(2)]
                for kk in range(KT):
                    ps = ps_s.tile([128, 2 * CH], F32, tag="s", name="ps_s")
                    for hh in range(2):  # head within pair
                        nc.tensor.matmul(
                            ps[:, bass.ts(hh, CH)],
                            kt[pr][bass.ds(64 * hh, 64), bass.ts(kk, 128)],
                            qt[pr][bass.ds(64 * hh, 64), bass.ds(CH * ch, CH)],
                            start=True, stop=True,
                            tile_position=(64 * hh, 0))
                    e16 = pe.tile([128, 2 * CH], F16, tag="E", name="e16")
                    nc.scalar.activation(e16[:], ps[:], EXP, scale=scale)
                    pt = ppp.tile([128, 2 * CH], F16, tag="P", name="pt")
                    nc.vector.tensor_mul(
                        pt[:].rearrange("p (h q) -> p h q", h=2),
                        e16[:].rearrange("p (h q) -> p h q", h=2),
                        mk[kk][:].unsqueeze(1).broadcast_to([128, 2, CH]))
                    if "dbg" in io and ch == 0 and pr == 0 and kk == 0:
                        nc.sync.dma_start(io["dbg_e"][:, :], e16[:])
                        nc.sync.dma_start(io["dbg_p"][:, :], pt[:])
                    for hh in range(2):
                        nc.tensor.matmul(
                            av[hh][:], v[:, kk, 2 * pr + hh, 0:65],
                            pt[:, bass.ts(hh, CH)],
                            start=(kk == 0), stop=(kk == KT - 1))
                for hh in range(2):
                    ysb = pys.tile([65, CH], F32, tag="ys", name="ysb")
                    nc.scalar.copy(ysb[:], av[hh][:])
                    if "dbg" in io and ch == 0 and pr == 0 and hh == 0:
                        nc.sync.dma_start(io["dbg_y"][:, :], ysb[:])
                    r0 = prr.tile([1, CH], F32, tag="r0", name="r0")
                    nc.vector.tensor_copy(r0[:], ysb[64:65, :])
                    rr32 = prr.tile([1, CH], F32, tag="rr32", name="rr32")
                    nc.vector.reciprocal_approx_fast(rr32[:], r0[:])
                    rr = prr.tile([1, CH], F16, tag="rr", name="rr")
                    with nc.allow_low_precision(reason="fp16 recip copy ok"):
                        nc.vector.tensor_copy(rr[:], rr32[:])
                    bc = ps_b.tile([64, CH], F32, tag="bc", name="bc")
                    nc.tensor.matmul(bc[:], ones[:], rr[:], start=True, stop=True)
                    nc.vector.tensor_mul(yts[pr][bass.ds(64 * hh, 64), :],
                                         ysb[0:64, :], bc[:])
            # out[q, :] = sum_e Y^T[e, q] * WuT[e, :] + bu
            for qi in range(CH // 128):
                pso = ps_av.tile([128, EMB], F32, tag="av", name="pso")
                for e in range(CT):
                    nc.tensor.matmul(pso[:], yts[e][:, bass.ts(qi, 128)], wu[e][:],
                                     start=(e == 0), stop=(e == CT - 1))
                osb = po.tile([128, EMB], F32, tag="o", name="osb")
                nc.vector.tensor_add(osb[:], pso[:], bub[:])
                nc.sync.dma_start(
                    io["out"][bass.ds(CH * ch + 128 * qi, 128), :], osb[:])


def build(cfg, num_devices=8, dbg=False):
    T, QC = cfg["T"], cfg["QC"]
    nc = bacc.Bacc("TRN2", target_bir_lowering=False, debug=False,
                   num_devices=num_devices)
    io = {
        "x1T": nc.dram_tensor("x1T", [EMB, QC], F16, kind="ExternalInput").ap(),
        "x2T": nc.dram_tensor("x2T", [EMB, T], F16, kind="ExternalInput").ap(),
        "maskT": nc.dram_tensor("maskT", [T, QC], F16, kind="ExternalInput").ap(),
        "wqT": nc.dram_tensor("wqT", [EMB, EMB], F16, kind="ExternalInput").ap(),
        "wkT": nc.dram_tensor("wkT", [EMB, EMB], F16, kind="ExternalInput").ap(),
        "wvT": nc.dram_tensor("wvT", [EMB, EMB], F16, kind="ExternalInput").ap(),
        "wuT": nc.dram_tensor("wuT", [EMB, EMB], F16, kind="ExternalInput").ap(),
        "bqr": nc.dram_tensor("bqr", [128, CT], F32, kind="ExternalInput").ap(),
        "bkr": nc.dram_tensor("bkr", [128, CT], F32, kind="ExternalInput").ap(),
        "bvb": nc.dram_tensor("bvb", [128, EMB], F32, kind="ExternalInput").ap(),
        "bub": nc.dram_tensor("bub", [128, EMB], F32, kind="ExternalInput").ap(),
        "out": nc.dram_tensor("out", [QC, EMB], F32, kind="ExternalOutput").ap(),
    }
    if dbg:
        io["dbg"] = True
        CH = min(512, QC)
        io["dbg_qt"] = nc.dram_tensor("dbg_qt", [EMB, QC], F16, kind="ExternalOutput").ap()
        io["dbg_kt"] = nc.dram_tensor("dbg_kt", [EMB, T], F16, kind="ExternalOutput").ap()
        io["dbg_v"] = nc.dram_tensor("dbg_v", [128, (T // 128) * H * 66], F16, kind="ExternalOutput").ap()
        io["dbg_e"] = nc.dram_tensor("dbg_e", [128, 2 * CH], F16, kind="ExternalOutput").ap()
        io["dbg_p"] = nc.dram_tensor("dbg_p", [128, 2 * CH], F16, kind="ExternalOutput").ap()
        io["dbg_y"] = nc.dram_tensor("dbg_y", [65, CH], F32, kind="ExternalOutput").ap()
    with tile.TileContext(nc) as tc:
        with ExitStack() as ctx:
            attention_body(ctx, tc, io, cfg)
    nc.compile()
    return nc


def host_prep(x1, x2, mask, Wq, bq, Wk, bk, Wv, bv, Wu, bu, cfg):
    """Build the 8 per-core input maps from full inputs."""
    T, QC = cfg["T"], cfg["QC"]
    shared = {
        "wqT": np.ascontiguousarray(Wq.T).astype(np.float16),
        "wkT": np.ascontiguousarray(Wk.T).astype(np.float16),
        "wvT": np.ascontiguousarray(Wv.T).astype(np.float16),
        "wuT": np.ascontiguousarray(Wu.T).astype(np.float16),
        "bqr": np.ascontiguousarray(bq.reshape(CT, 128).T).astype(np.float32),
        "bkr": np.ascontiguousarray(bk.reshape(CT, 128).T).astype(np.float32),
        "bvb": np.ascontiguousarray(np.broadcast_to(bv, (128, EMB))).astype(np.float32),
        "bub": np.ascontiguousarray(np.broadcast_to(bu, (128, EMB))).astype(np.float32),
    }
    x2T = [x2[b].T.astype(np.float16) for b in range(x1.shape[0])]
    in_maps = []
    n_cores = (x1.shape[0] * x1.shape[1]) // QC
    per_b = x1.shape[1] // QC
    for c in range(n_cores):
        b, q0 = c // per_b, (c % per_b) * QC
        in_maps.append(dict(
            shared,
            x1T=x1[b, q0:q0 + QC, :].T.astype(np.float16),
            x2T=x2T[b],
            maskT=mask[b, q0:q0 + QC, :].T.astype(np.float16),
        ))
    return in_maps


_NC_CACHE = {}


def kernel(x1, x2, mask, Wq, bq, Wk, bk, Wv, bv, Wu, bu):
    cfg = FULL_CFG
    B, TQ, _ = x1.shape
    in_maps = host_prep(np.asarray(x1, np.float32), np.asarray(x2, np.float32),
                        np.asarray(mask), np.asarray(Wq, np.float32),
                        np.asarray(bq, np.float32), np.asarray(Wk, np.float32),
                        np.asarray(bk, np.float32), np.asarray(Wv, np.float32),
                        np.asarray(bv, np.float32), np.asarray(Wu, np.float32),
                        np.asarray(bu, np.float32), cfg)
    key = (cfg["T"], cfg["QC"])
    if key not in _NC_CACHE:
        _NC_CACHE[key] = build(cfg)
    nc = _NC_CACHE[key]
    res = run_bass_kernel_spmd(nc, in_maps, core_ids=list(range(8)),
                               trace=bool(os.environ.get("KERNEL_TRACE")))
    if os.environ.get("KERNEL_TRACE"):
        kernel.last_exec_ns = res.exec_time_ns
        kernel.last_results = res
    out = np.empty((B, TQ, EMB), np.float32)
    per_b = TQ // cfg["QC"]
    for c in range(8):
        b, q0 = c // per_b, (c % per_b) * cfg["QC"]
        out[b, q0:q0 + cfg["QC"], :] = res.results[c]["out"]
    return out
